# revision 10
# baseline (speedup 1.0000x reference)
"""Multi-head causal attention (B=4, T=2048, H=16, D=64) on 8 trn2 NeuronCores.

Sharding: core c = (batch b = c//2, head-group hg = c%2 of 8 heads).
Each core computes its batch's QKV projection for its 8 heads, causal
attention, and a partial output projection (contraction over its 512
channels of W_proj). Host sums the two partials per batch and adds bias.

Per-core layout (inherited from the fp16 baseline):
  - x is passed pre-transposed as xT [C=1024, T=2048] (fp16).
  - K^T, Q^T stored [hd, t] with head-dim on partitions (64 per head, 2
    heads per 128-partition tile) -> scores matmuls row-pack 2 heads.
  - V stored [t, h*65+d] with a ones column appended per head -> the AV
    matmul O_T = V_aug^T(stationary) x P_T produces softmax denominators
    in row 64 for free.
  - Scores are computed transposed S_T[k, q] so that P_T = exp(S_T) is
    directly the AV matmul's moving operand; softmax needs no max
    subtraction (|scores/8| < ~3) and no separate sum reduction.
  - O_T [hd, t] is exactly the lhsT the output projection needs.

Schedule: the attention phase is ACT(exp)-throughput bound (~1.15us per
128x(2x512) k-tile vs ~0.85us of PE work), and the Tile scheduler bakes
its instruction order at compile time, so QKV/output-projection matmuls
are STATICALLY interleaved one-at-a-time between attention k-tiles (a
filler queue) with the remainder flushed between attention phases.
Dedicated PSUM pools (scores 2x2 banks / AV accum 2x1 / gemm 2x1) keep
attention and gemm from starving each other.  Diagonal k-tiles run
first in each pair so the DVE mask-multiply sits in the pipeline fill.
The AV accumulators are copied psum->sbuf immediately after the last AV
so the next pair's AVs aren't gated on the (slow, gpsimd-broadcast)
normalize chain.  Boot DMAs are one-contraction-tile-sized (0.125 MB)
across all 16 queues so the first matmul waits ~3us, not 13.  y is
fp16 (halves output DMA; host sums partials in f32).
"""

import os
import sys

import numpy as np

F16_NP = np.dtype(np.float16)

if "/opt/trn_rl_repo" not in sys.path:
    sys.path.insert(0, "/opt/trn_rl_repo")

from collections import deque
from contextlib import ExitStack

import concourse.bass as bass
import concourse.bacc as bacc
import concourse.mybir as mybir
import concourse.tile as tile
from concourse._compat import with_exitstack

P = 128
T = 2048
C = 1024
H_PER_CORE = 8
D = 64
DP = D + 1  # V augmented with a ones column
NC_CORES = 8

TB = 4  # t-blocks of 512
QB = 4  # q-blocks of 512
CI = 8  # contraction tiles of 128 over C for QKV proj

F32 = mybir.dt.float32
F16 = mybir.dt.float16  # fp16: full matmul rate, 8x finer mantissa than bf16


@with_exitstack
def build_attention_kernel(ctx: ExitStack, tc: tile.TileContext):
    nc = tc.nc

    xT = nc.declare_dram_parameter("xT", [C, T], F16, isOutput=False)
    wk = nc.declare_dram_parameter("wk", [C, 512], F16, isOutput=False)
    wq = nc.declare_dram_parameter("wq", [C, 512], F16, isOutput=False)
    wv = nc.declare_dram_parameter("wv", [C, 512], F16, isOutput=False)
    wp = nc.declare_dram_parameter("wp", [512, C], F16, isOutput=False)
    y = nc.declare_dram_parameter("y", [T, C], F16, isOutput=True)

    xT_t = xT.rearrange("(co ci) t -> ci co t", ci=P)
    wk_t = wk.rearrange("(co ci) m -> ci co m", ci=P)
    wq_t = wq.rearrange("(co ci) m -> ci co m", ci=P)
    wv_t = wv.rearrange("(co ci) m -> ci co m", ci=P)
    wp_t = wp.rearrange("(co ci) n -> ci co n", ci=P)
    y_t = y.rearrange("(tt p) n -> p tt n", p=P)

    # ---- SBUF pools ----
    kt_pool = ctx.enter_context(tc.tile_pool(name="ktp", bufs=16))
    qt_pool = ctx.enter_context(tc.tile_pool(name="qtp", bufs=16))
    ot_pool = ctx.enter_context(tc.tile_pool(name="otp", bufs=16))
    v_pool = ctx.enter_context(tc.tile_pool(name="vp", bufs=4))
    const_pool = ctx.enter_context(tc.tile_pool(name="constp", bufs=1))
    w_pool = ctx.enter_context(tc.tile_pool(name="wp_", bufs=1))
    pt_pool = ctx.enter_context(tc.tile_pool(name="ptp", bufs=8))
    osb_pool = ctx.enter_context(tc.tile_pool(name="osbp", bufs=8))
    recip_pool = ctx.enter_context(tc.tile_pool(name="recipp", bufs=6))
    bc_pool = ctx.enter_context(tc.tile_pool(name="bcp", bufs=4))
    y_pool = ctx.enter_context(tc.tile_pool(name="yp", bufs=2))
    # ---- PSUM pools: 4 banks scores, 2 banks AV accum, 2 banks gemm ----
    ps_s_pool = ctx.enter_context(tc.tile_pool(name="ps_s", bufs=2, space="PSUM"))
    ps_o_pool = ctx.enter_context(tc.tile_pool(name="ps_o", bufs=2, space="PSUM"))
    ps_g_pool = ctx.enter_context(tc.tile_pool(name="ps_g", bufs=2, space="PSUM"))

    # KT[pt][tb], QT[pt][qb]: [128, 512]; partitions = 2 heads x 64 dims
    KT = [[kt_pool.tile([P, 512], F16, tag="kt", name=f"KT_{pt}_{tb}") for tb in range(TB)] for pt in range(4)]
    QT = [[qt_pool.tile([P, 512], F16, tag="qt", name=f"QT_{pt}_{qb}") for qb in range(QB)] for pt in range(4)]
    OT = [[ot_pool.tile([P, 512], F16, tag="ot", name=f"OT_{hp}_{qb}") for qb in range(QB)] for hp in range(4)]
    V = [v_pool.tile([P, 4, H_PER_CORE * DP], F16, tag="v", name=f"V_{tb}") for tb in range(TB)]
    masks = const_pool.tile([P, 4, 512], F16, tag="masks", name="masks")
    warm = const_pool.tile([1, 8], F32, tag="warm", name="warm")
    # weights + x^T chunked per contraction tile for fine-grained DMA deps
    wk_c = [w_pool.tile([P, 512], F16, name=f"wk_{i}") for i in range(CI)]
    wq_c = [w_pool.tile([P, 4, 512], F16, name=f"wq_{i}") for i in range(2)]
    wv_c = [w_pool.tile([P, 4, 512], F16, name=f"wv_{i}") for i in range(2)]
    wp_c = w_pool.tile([P, 4, C], F16, name="wp_c")
    xt0 = [w_pool.tile([P, 512], F16, name=f"xt0_{ci}") for ci in range(CI)]
    xtb = [w_pool.tile([P, CI, 512], F16, name=f"xt_{tb}") for tb in range(1, TB)]

    # warm up the ACT exp table (ACT_TABLE_LOAD ~2.7us) during the prologue
    nc.gpsimd.memset(warm[:], 0.0)
    nc.scalar.activation(warm[:], warm[:], mybir.ActivationFunctionType.Exp)

    # diagonal causal masks: masks[:, j, :][kk, qq] = 1.0 if qq >= kk + j*128
    for j in range(4):
        nc.gpsimd.memset(masks[:, j, :], 1.0)
        nc.gpsimd.affine_select(
            out=masks[:, j, :],
            in_=masks[:, j, :],
            compare_op=mybir.AluOpType.is_ge,
            fill=0.0,
            base=-j * P,
            pattern=[[1, 512]],
            channel_multiplier=-1,
        )
    # ones column of V
    for tb in range(TB):
        ones_col = V[tb].rearrange("p s (h e) -> p s h e", e=DP)[:, :, :, D : D + 1]
        nc.gpsimd.memset(ones_col, 1.0)

    # ---- boot DMAs.  Each dma_start costs ~0.6us of SERIAL issue time on
    # its (HWDGE) engine, so the first wave (wk + x block 0, per-ci chunks)
    # alternates between the sync and scalar sequencers, the mid-priority
    # weights go on sync, and the late x blocks go through gpsimd SWDGE. ----
    for ci in range(CI):
        eng = nc.sync if ci % 2 == 0 else nc.scalar
        eng2 = nc.scalar if ci % 2 == 0 else nc.sync
        eng.dma_start(wk_c[ci][:], wk_t[:, ci])
        eng2.dma_start(xt0[ci][:], xT_t[:, ci, 0:512])
    for i in range(2):
        nc.sync.dma_start(wq_c[i][:], wq_t[:, 4 * i : 4 * i + 4])
        nc.sync.dma_start(wv_c[i][:], wv_t[:, 4 * i : 4 * i + 4])
    nc.sync.dma_start(xtb[0][:], xT_t[:, :, 512:1024])
    nc.gpsimd.dma_start(xtb[1][:], xT_t[:, :, 1024:1536])
    nc.gpsimd.dma_start(xtb[2][:], xT_t[:, :, 1536:2048])
    nc.gpsimd.dma_start(wp_c[:], wp_t[:, :])

    # ---- gemm work units (one matmul or copy each), interleaved into the
    # ACT-bound attention phases via take_filler ----
    def x_slice(tb, ci, cols):
        if tb == 0:
            return xt0[ci][:, cols]
        return xtb[tb - 1][:, ci, cols]

    qkv_fillers = deque()
    carry_fillers = deque()
    toggle = [0]

    def take_filler(n):
        for _ in range(n):
            toggle[0] ^= 1
            order = (
                (qkv_fillers, carry_fillers)
                if toggle[0]
                else (carry_fillers, qkv_fillers)
            )
            for q in order:
                if q:
                    q.popleft()()
                    break

    def flush_qkv():
        while qkv_fillers:
            qkv_fillers.popleft()()

    def flush_all():
        flush_qkv()
        while carry_fillers:
            carry_fillers.popleft()()

    def qkv_group(kind, tb, idx):
        """One [128,512] psum group of the QKV projection; returns units."""
        state = {}

        def mm(ci):
            def emit():
                if ci == 0:
                    state["ps"] = ps_g_pool.tile([P, 512], F32, tag="g_ps", name="g_ps")
                if kind == "v":
                    lhsT = x_slice(tb, ci, slice(idx * P, (idx + 1) * P))
                    rhs = wv_c[ci // 4][:, ci % 4, :]
                elif kind == "k":
                    lhsT = wk_c[ci][:, idx * P : (idx + 1) * P]
                    rhs = x_slice(tb, ci, slice(None))
                else:
                    lhsT = wq_c[ci // 4][:, ci % 4, idx * P : (idx + 1) * P]
                    rhs = x_slice(tb, ci, slice(None))
                nc.tensor.matmul(
                    state["ps"][:], lhsT=lhsT, rhs=rhs, start=(ci == 0), stop=(ci == CI - 1)
                )

            return emit

        def cp():
            ps = state["ps"]
            if kind == "k":
                nc.vector.tensor_copy(KT[idx][tb][:], ps[:])
            elif kind == "q":
                nc.vector.tensor_copy(QT[idx][tb][:], ps[:])
            else:
                nc.vector.tensor_copy(
                    V[tb][:, idx].rearrange("p (h e) -> p h e", e=DP)[:, :, :D],
                    ps.rearrange("p (h d) -> p h d", d=D),
                )

        return [mm(ci) for ci in range(CI)] + [cp]

    def push_qkv(tb):
        """Order K0,Q0,V*,K1,Q1,... unblocks attention pair hp=0 earliest."""
        order = [("k", 0), ("q", 0), ("v", 0), ("v", 1), ("v", 2), ("v", 3)]
        order += [(kq, pt) for pt in range(1, 4) for kq in ("k", "q")]
        for kind, idx in order:
            qkv_fillers.extend(qkv_group(kind, tb, idx))

    ysbs = {}

    def proj_tile_units(tt, pair_pool):
        """Output-projection t-tile tt; 'pair_pool' uses one [128,2,512]
        scores-pool tile for both halves (tail tiles), else two gemm tiles."""
        qb, sub = tt // 4, tt % 4
        state = {}

        def mm(nb, ct):
            def emit():
                if (nb, ct) == (0, 0):
                    if tt % 2 == 0 or (tt // 2) not in ysbs:
                        ysbs[tt // 2] = y_pool.tile([P, 2, C], F16, tag="ypair", name="ypair")
                    if pair_pool:
                        state["ps"] = ps_s_pool.tile([P, 2, 512], F32, tag="s_ps", name="y_ps")
                    else:
                        state["ps"] = [
                            ps_g_pool.tile([P, 512], F32, tag="g_ps", name="y_ps")
                            for _ in range(2)
                        ]
                ps = state["ps"][:, nb, :] if pair_pool else state["ps"][nb][:]
                nc.tensor.matmul(
                    ps,
                    lhsT=OT[ct][qb][:, sub * P : (sub + 1) * P],
                    rhs=wp_c[:, ct, nb * 512 : (nb + 1) * 512],
                    start=(ct == 0),
                    stop=(ct == 3),
                )

            return emit

        def cp_dma():
            ysb = ysbs[tt // 2]
            dst = ysb[:, tt % 2, :].rearrange("p (b n) -> p b n", b=2)
            if pair_pool:
                nc.vector.tensor_copy(dst, state["ps"][:])
            else:
                for nb in range(2):
                    nc.vector.tensor_copy(dst[:, nb, :], state["ps"][nb][:])
            nc.sync.dma_start(y_t[:, tt : tt + 1, :], ysb[:, tt % 2 : tt % 2 + 1, :])

        units = [mm(nb, ct) for nb in range(2) for ct in range(4)]
        units.append(cp_dma)
        return units

    def push_proj(qb):
        for tt in range(4 * qb, 4 * qb + 4):
            carry_fillers.extend(proj_tile_units(tt, pair_pool=False))

    def attention_pair(qb, hp, eager_normalize=False):
        ot_ps = [ps_o_pool.tile([DP, 512], F32, tag="ot_ps", name=f"ot_ps_{i}") for i in range(2)]
        nkt = 4 * (qb + 1)
        # diagonal k-tiles first: the DVE mask-multiply latency lands in the
        # pipeline fill instead of the steady state
        kt_order = list(range(4 * qb, nkt)) + list(range(0, 4 * qb))
        pts = {}

        def emit_scores_exp(kt):
            tb = kt // 4
            qs = (kt - 4 * qb) * P if kt >= 4 * qb else 0
            nq = 512 - qs
            s_ps = ps_s_pool.tile([P, 2, 512], F32, tag="s_ps", name="s_ps")
            for h2 in range(2):
                # S_T[k, q] for head h = 2*hp + h2 (row-packed pair)
                nc.tensor.matmul(
                    s_ps[:, h2, qs:],
                    lhsT=KT[hp][tb][
                        h2 * D : (h2 + 1) * D,
                        (kt % 4) * P : (kt % 4 + 1) * P,
                    ],
                    rhs=QT[hp][qb][h2 * D : (h2 + 1) * D, qs:],
                    start=True,
                    stop=True,
                )
            p_t = pt_pool.tile([P, 2, 512], F16, tag="pt", name="p_t")
            nc.scalar.activation(
                p_t[:, :, qs:],
                s_ps[:, :, qs:],
                mybir.ActivationFunctionType.Exp,
                scale=0.125,
            )
            if kt >= 4 * qb:  # diagonal: zero q < k entries
                j = kt - 4 * qb
                mb = masks[:, j : j + 1, qs:].to_broadcast([P, 2, nq])
                nc.vector.tensor_mul(p_t[:, :, qs:], p_t[:, :, qs:], mb)
            pts[kt] = (p_t, qs)

        def emit_av(kt, first, last):
            tb = kt // 4
            p_t, qs = pts.pop(kt)
            for h2 in range(2):
                h = 2 * hp + h2
                nc.tensor.matmul(
                    ot_ps[h2][:, qs:],
                    lhsT=V[tb][:, kt % 4, h * DP : (h + 1) * DP],
                    rhs=p_t[:, h2, qs:],
                    start=first,
                    stop=last,
                )

        # software pipeline: S(i+1) before AV(i), one gemm filler per k-tile
        emit_scores_exp(kt_order[0])
        for i in range(1, nkt):
            emit_scores_exp(kt_order[i])
            emit_av(kt_order[i - 1], first=(i == 1), last=False)
            take_filler(1)
        emit_av(kt_order[-1], first=(nkt == 1), last=True)

        # release the AV psum banks immediately; normalize off the critical
        # path from the sbuf copy
        osb = [osb_pool.tile([DP, 512], F32, tag="osb", name="osb") for _ in range(2)]
        for h2 in range(2):
            nc.vector.tensor_copy(osb[h2][:], ot_ps[h2][:])

        def norm_unit(h2):
            def emit():
                recip = recip_pool.tile([1, 512], F32, tag="recip", name="recip")
                nc.vector.tensor_copy(recip[:], osb[h2][D : D + 1, :])
                nc.vector.reciprocal_approx_fast(recip[:], recip[:])
                bc = bc_pool.tile([D, 512], F32, tag="bc", name="bc")
                nc.gpsimd.partition_broadcast(bc[:], recip[:])
                nc.vector.tensor_mul(
                    OT[hp][qb][h2 * D : (h2 + 1) * D, :],
                    osb[h2][:D, :],
                    bc[:],
                )

            return emit

        # normalize is deferred through the filler queue so its DVE/gpsimd
        # chain doesn't sit in front of the next pair's mask-multiplies;
        # the final pairs run it eagerly (the output-proj tail gates on OT)
        if eager_normalize:
            for h2 in range(2):
                norm_unit(h2)()
        else:
            for h2 in range(2):
                carry_fillers.append(norm_unit(h2))

    # ---- schedule ----
    # prologue: qkv(0) straight (nothing to overlap with yet)
    push_qkv(0)
    flush_qkv()
    for qb in range(QB):
        if qb < 3:
            push_qkv(qb + 1)
        if qb >= 1:
            push_proj(qb - 1)
        for hp in range(4):
            attention_pair(qb, hp, eager_normalize=(qb == 3 and hp >= 2))
            take_filler(2)
        flush_qkv()
    flush_all()
    # tail: last 4 proj tiles with 4 psum tiles in flight so only the ct=3
    # matmuls wait on the last pair's normalize
    tail_units = [proj_tile_units(tt, pair_pool=(tt < 14)) for tt in range(12, 16)]
    for tt_i in range(3):  # ct 0..2 of both halves for tiles 12,13,14
        for u in (0, 1, 2, 4, 5, 6):
            tail_units[tt_i][u]()
    for tt_i in range(3):  # ct=3 closes + copy + dma
        tail_units[tt_i][3]()
        tail_units[tt_i][7]()
        tail_units[tt_i][8]()
    for u in tail_units[3]:
        u()

    return nc


_CACHED_NC = None


def get_nc():
    global _CACHED_NC
    if _CACHED_NC is None:
        nc = bacc.Bacc()
        with tile.TileContext(nc) as tc:
            build_attention_kernel(tc)
        nc.compile()
        _CACHED_NC = nc
    return _CACHED_NC


def make_in_maps(x, W_att, W_proj):
    x = np.asarray(x, dtype=np.float32)
    W_att = np.asarray(W_att, dtype=np.float32)
    in_maps = []
    for c in range(NC_CORES):
        b, hg = c // 2, c % 2
        s = hg * 512
        in_maps.append(
            {
                "xT": np.ascontiguousarray(x[b].T).astype(F16_NP),
                "wk": np.ascontiguousarray(
                    W_att[:, 0 * C + s : 0 * C + s + 512]
                ).astype(F16_NP),
                "wq": np.ascontiguousarray(
                    W_att[:, 1 * C + s : 1 * C + s + 512]
                ).astype(F16_NP),
                "wv": np.ascontiguousarray(
                    W_att[:, 2 * C + s : 2 * C + s + 512]
                ).astype(F16_NP),
                "wp": np.ascontiguousarray(
                    np.asarray(W_proj, np.float32)[s : s + 512]
                ).astype(F16_NP),
            }
        )
    return in_maps


def combine_outputs(results, b_proj):
    B = NC_CORES // 2
    out = np.empty((B, T, C), dtype=np.float32)
    bias = np.asarray(b_proj, dtype=np.float32)
    for b in range(B):
        out[b] = (
            results[2 * b]["y"].astype(np.float32)
            + results[2 * b + 1]["y"].astype(np.float32)
            + bias
        )
    return out


def kernel(x, W_att, W_proj, b_proj):
    from concourse.bass_utils import run_bass_kernel_spmd

    nc = get_nc()
    in_maps = make_in_maps(x, W_att, W_proj)
    res = run_bass_kernel_spmd(nc, in_maps, list(range(NC_CORES)))
    return combine_outputs(res.results, b_proj)


# revision 11
# speedup vs baseline: 1.0559x; 1.0559x over previous
"""Multi-head causal attention (B=4, T=2048, H=16, D=64) on 8 trn2 NeuronCores.

Sharding: core c = (batch b = c//2, head-group hg = c%2 of 8 heads).
Each core computes its batch's QKV projection for its 8 heads, causal
attention, and a partial output projection (contraction over its 512
channels of W_proj). Host sums the two partials per batch and adds bias.

Per-core layout (inherited from the fp16 baseline):
  - x is passed pre-transposed as xT [C=1024, T=2048] (fp16).
  - K^T, Q^T stored [hd, t] with head-dim on partitions (64 per head, 2
    heads per 128-partition tile) -> scores matmuls row-pack 2 heads.
  - V stored [t, h*65+d] with a ones column appended per head -> the AV
    matmul O_T = V_aug^T(stationary) x P_T produces softmax denominators
    in row 64 for free.
  - Scores are computed transposed S_T[k, q] so that P_T = exp(S_T) is
    directly the AV matmul's moving operand; softmax needs no max
    subtraction (|scores/8| < ~3) and no separate sum reduction.
  - O_T [hd, t] is exactly the lhsT the output projection needs.

Schedule: the attention phase is ACT(exp)-throughput bound (~1.15us per
128x(2x512) k-tile vs ~0.85us of PE work), and the Tile scheduler bakes
its instruction order at compile time, so QKV/output-projection matmuls
are STATICALLY interleaved one-at-a-time between attention k-tiles (a
filler queue) with the remainder flushed between attention phases.
Dedicated PSUM pools (scores 2x2 banks / AV accum 2x1 / gemm 2x1) keep
attention and gemm from starving each other.  Diagonal k-tiles run
first in each pair so the DVE mask-multiply sits in the pipeline fill.
The AV accumulators are copied psum->sbuf immediately after the last AV
so the next pair's AVs aren't gated on the (slow, gpsimd-broadcast)
normalize chain.  Boot DMAs are one-contraction-tile-sized (0.125 MB)
across all 16 queues so the first matmul waits ~3us, not 13.  y is
fp16 (halves output DMA; host sums partials in f32).
"""

import os
import sys

import numpy as np

F16_NP = np.dtype(np.float16)

if "/opt/trn_rl_repo" not in sys.path:
    sys.path.insert(0, "/opt/trn_rl_repo")

from collections import deque
from contextlib import ExitStack

import concourse.bass as bass
import concourse.bacc as bacc
import concourse.mybir as mybir
import concourse.tile as tile
from concourse._compat import with_exitstack

P = 128
T = 2048
C = 1024
H_PER_CORE = 8
D = 64
DP = D + 1  # V augmented with a ones column
NC_CORES = 8

TB = 4  # t-blocks of 512
QB = 4  # q-blocks of 512
CI = 8  # contraction tiles of 128 over C for QKV proj

F32 = mybir.dt.float32
F16 = mybir.dt.float16  # fp16: full matmul rate, 8x finer mantissa than bf16


@with_exitstack
def build_attention_kernel(ctx: ExitStack, tc: tile.TileContext):
    nc = tc.nc

    xT = nc.declare_dram_parameter("xT", [C, T], F16, isOutput=False)
    wk = nc.declare_dram_parameter("wk", [C, 512], F16, isOutput=False)
    wq = nc.declare_dram_parameter("wq", [C, 512], F16, isOutput=False)
    wv = nc.declare_dram_parameter("wv", [C, 512], F16, isOutput=False)
    wp = nc.declare_dram_parameter("wp", [512, C], F16, isOutput=False)
    y = nc.declare_dram_parameter("y", [T, C], F16, isOutput=True)

    xT_t = xT.rearrange("(co ci) t -> ci co t", ci=P)
    wk_t = wk.rearrange("(co ci) m -> ci co m", ci=P)
    wq_t = wq.rearrange("(co ci) m -> ci co m", ci=P)
    wv_t = wv.rearrange("(co ci) m -> ci co m", ci=P)
    wp_t = wp.rearrange("(co ci) n -> ci co n", ci=P)
    y_t = y.rearrange("(tt p) n -> p tt n", p=P)

    # ---- SBUF pools ----
    kt_pool = ctx.enter_context(tc.tile_pool(name="ktp", bufs=16))
    qt_pool = ctx.enter_context(tc.tile_pool(name="qtp", bufs=16))
    ot_pool = ctx.enter_context(tc.tile_pool(name="otp", bufs=16))
    v_pool = ctx.enter_context(tc.tile_pool(name="vp", bufs=4))
    const_pool = ctx.enter_context(tc.tile_pool(name="constp", bufs=1))
    w_pool = ctx.enter_context(tc.tile_pool(name="wp_", bufs=1))
    pt_pool = ctx.enter_context(tc.tile_pool(name="ptp", bufs=8))
    osb_pool = ctx.enter_context(tc.tile_pool(name="osbp", bufs=8))
    recip_pool = ctx.enter_context(tc.tile_pool(name="recipp", bufs=6))
    bc_pool = ctx.enter_context(tc.tile_pool(name="bcp", bufs=4))
    y_pool = ctx.enter_context(tc.tile_pool(name="yp", bufs=2))
    # ---- PSUM pools: 4 banks scores, 2 banks AV accum, 2 banks gemm ----
    ps_s_pool = ctx.enter_context(tc.tile_pool(name="ps_s", bufs=2, space="PSUM"))
    ps_o_pool = ctx.enter_context(tc.tile_pool(name="ps_o", bufs=2, space="PSUM"))
    ps_g_pool = ctx.enter_context(tc.tile_pool(name="ps_g", bufs=2, space="PSUM"))

    # KT[pt][tb], QT[pt][qb]: [128, 512]; partitions = 2 heads x 64 dims
    KT = [[kt_pool.tile([P, 512], F16, tag="kt", name=f"KT_{pt}_{tb}") for tb in range(TB)] for pt in range(4)]
    QT = [[qt_pool.tile([P, 512], F16, tag="qt", name=f"QT_{pt}_{qb}") for qb in range(QB)] for pt in range(4)]
    OT = [[ot_pool.tile([P, 512], F16, tag="ot", name=f"OT_{hp}_{qb}") for qb in range(QB)] for hp in range(4)]
    V = [v_pool.tile([P, 4, H_PER_CORE * DP], F16, tag="v", name=f"V_{tb}") for tb in range(TB)]
    masks = const_pool.tile([P, 4, 512], F16, tag="masks", name="masks")
    warm = const_pool.tile([1, 8], F32, tag="warm", name="warm")
    # weights + x^T chunked per contraction tile for fine-grained DMA deps
    wk_c = [w_pool.tile([P, 512], F16, name=f"wk_{i}") for i in range(CI)]
    wq_c = [w_pool.tile([P, 4, 512], F16, name=f"wq_{i}") for i in range(2)]
    wv_c = [w_pool.tile([P, 4, 512], F16, name=f"wv_{i}") for i in range(2)]
    wp_c = w_pool.tile([P, 4, C], F16, name="wp_c")
    xt0 = [w_pool.tile([P, 512], F16, name=f"xt0_{ci}") for ci in range(CI)]
    xtb = [w_pool.tile([P, CI, 512], F16, name=f"xt_{tb}") for tb in range(1, TB)]

    # late x blocks + wp via gpsimd SWDGE, issued before its memset work
    nc.gpsimd.dma_start(xtb[1][:], xT_t[:, :, 1024:1536])
    nc.gpsimd.dma_start(xtb[2][:], xT_t[:, :, 1536:2048])
    nc.gpsimd.dma_start(wp_c[:], wp_t[:, :])
    # ones column of V (needed by the first V-projection copies)
    for tb in range(TB):
        ones_col = V[tb].rearrange("p s (h e) -> p s h e", e=DP)[:, :, :, D : D + 1]
        nc.gpsimd.memset(ones_col, 1.0)
    nc.gpsimd.memset(warm[:], 0.0)

    # diagonal causal masks: masks[:, j, :][kk, qq] = 1.0 if qq >= kk + j*128
    for j in range(4):
        nc.gpsimd.memset(masks[:, j, :], 1.0)
        nc.gpsimd.affine_select(
            out=masks[:, j, :],
            in_=masks[:, j, :],
            compare_op=mybir.AluOpType.is_ge,
            fill=0.0,
            base=-j * P,
            pattern=[[1, 512]],
            channel_multiplier=-1,
        )
    # ---- boot DMAs.  Each dma_start costs ~0.6us of SERIAL issue time on
    # its (HWDGE) engine, so the first wave (wk + x block 0, per-ci chunks)
    # alternates between the sync and scalar sequencers, the mid-priority
    # weights go on sync, and the late x blocks go through gpsimd SWDGE. ----
    for ci in range(CI):
        eng = nc.sync if ci % 2 == 0 else nc.scalar
        eng2 = nc.scalar if ci % 2 == 0 else nc.sync
        eng.dma_start(wk_c[ci][:], wk_t[:, ci])
        eng2.dma_start(xt0[ci][:], xT_t[:, ci, 0:512])
    for i in range(2):
        nc.sync.dma_start(wq_c[i][:], wq_t[:, 4 * i : 4 * i + 4])
        nc.sync.dma_start(wv_c[i][:], wv_t[:, 4 * i : 4 * i + 4])
    nc.sync.dma_start(xtb[0][:], xT_t[:, :, 512:1024])
    # warm up the ACT exp table (ACT_TABLE_LOAD ~2.7us) after the scalar
    # engine's DMA-issue work, while the first K matmuls run
    nc.scalar.activation(warm[:], warm[:], mybir.ActivationFunctionType.Exp)

    # ---- gemm work units (one matmul or copy each), interleaved into the
    # ACT-bound attention phases via take_filler ----
    def x_slice(tb, ci, cols):
        if tb == 0:
            return xt0[ci][:, cols]
        return xtb[tb - 1][:, ci, cols]

    fillers = deque()  # entries: (is_qkv, closure)
    qkv_pending = [0]

    def take_filler(n):
        for _ in range(min(n, len(fillers))):
            is_qkv, fn = fillers.popleft()
            if is_qkv:
                qkv_pending[0] -= 1
            fn()

    def flush_qkv():
        # run units in FIFO order until all pending qkv-projection units
        # (needed by the next attention phase) have executed; later proj/
        # normalize units carry over into the next phase as fillers
        while qkv_pending[0] > 0:
            is_qkv, fn = fillers.popleft()
            if is_qkv:
                qkv_pending[0] -= 1
            fn()

    def flush_all():
        while fillers:
            fillers.popleft()[1]()
        qkv_pending[0] = 0

    def qkv_group(kind, tb, idx):
        """One [128,512] psum group of the QKV projection; returns units."""
        state = {}

        def mm(ci):
            def emit():
                if ci == 0:
                    state["ps"] = ps_g_pool.tile([P, 512], F32, tag="g_ps", name="g_ps")
                if kind == "v":
                    lhsT = x_slice(tb, ci, slice(idx * P, (idx + 1) * P))
                    rhs = wv_c[ci // 4][:, ci % 4, :]
                elif kind == "k":
                    lhsT = wk_c[ci][:, idx * P : (idx + 1) * P]
                    rhs = x_slice(tb, ci, slice(None))
                else:
                    lhsT = wq_c[ci // 4][:, ci % 4, idx * P : (idx + 1) * P]
                    rhs = x_slice(tb, ci, slice(None))
                nc.tensor.matmul(
                    state["ps"][:], lhsT=lhsT, rhs=rhs, start=(ci == 0), stop=(ci == CI - 1)
                )

            return emit

        def cp():
            ps = state["ps"]
            if kind == "k":
                nc.vector.tensor_copy(KT[idx][tb][:], ps[:])
            elif kind == "q":
                nc.vector.tensor_copy(QT[idx][tb][:], ps[:])
            else:
                nc.vector.tensor_copy(
                    V[tb][:, idx].rearrange("p (h e) -> p h e", e=DP)[:, :, :D],
                    ps.rearrange("p (h d) -> p h d", d=D),
                )

        return [mm(ci) for ci in range(CI)] + [cp]

    def push_qkv(tb):
        """Order K0,Q0,V*,K1,Q1,... unblocks attention pair hp=0 earliest.
        For the prologue block the wq/wv DMAs land ~2us after wk, so K0,K1
        run while they arrive."""
        if tb == 0:
            order = [("k", 0), ("k", 1), ("q", 0), ("q", 1)]
            order += [("v", i) for i in range(4)]
            order += [(kq, pt) for pt in range(2, 4) for kq in ("k", "q")]
        else:
            order = [("k", 0), ("q", 0), ("v", 0), ("v", 1), ("v", 2), ("v", 3)]
            order += [(kq, pt) for pt in range(1, 4) for kq in ("k", "q")]
        for kind, idx in order:
            units = qkv_group(kind, tb, idx)
            fillers.extend((True, u) for u in units)
            qkv_pending[0] += len(units)

    ysbs = {}

    def proj_tile_units(tt, pair_pool):
        """Output-projection t-tile tt; 'pair_pool' uses one [128,2,512]
        scores-pool tile for both halves (tail tiles), else two gemm tiles."""
        qb, sub = tt // 4, tt % 4
        state = {}

        def mm(nb, ct):
            def emit():
                if (nb, ct) == (0, 0):
                    if tt % 2 == 0 or (tt // 2) not in ysbs:
                        ysbs[tt // 2] = y_pool.tile([P, 2, C], F16, tag="ypair", name="ypair")
                    if pair_pool:
                        state["ps"] = ps_s_pool.tile([P, 2, 512], F32, tag="s_ps", name="y_ps")
                    else:
                        state["ps"] = [
                            ps_g_pool.tile([P, 512], F32, tag="g_ps", name="y_ps")
                            for _ in range(2)
                        ]
                ps = state["ps"][:, nb, :] if pair_pool else state["ps"][nb][:]
                nc.tensor.matmul(
                    ps,
                    lhsT=OT[ct][qb][:, sub * P : (sub + 1) * P],
                    rhs=wp_c[:, ct, nb * 512 : (nb + 1) * 512],
                    start=(ct == 0),
                    stop=(ct == 3),
                )

            return emit

        def cp_dma():
            ysb = ysbs[tt // 2]
            dst = ysb[:, tt % 2, :].rearrange("p (b n) -> p b n", b=2)
            if pair_pool:
                nc.vector.tensor_copy(dst, state["ps"][:])
            else:
                for nb in range(2):
                    nc.vector.tensor_copy(dst[:, nb, :], state["ps"][nb][:])
            nc.sync.dma_start(y_t[:, tt : tt + 1, :], ysb[:, tt % 2 : tt % 2 + 1, :])

        units = [mm(nb, ct) for nb in range(2) for ct in range(4)]
        units.append(cp_dma)
        return units

    def push_proj(qb):
        for tt in range(4 * qb, 4 * qb + 4):
            fillers.extend((False, u) for u in proj_tile_units(tt, pair_pool=False))

    def attention_pair(qb, hp, eager_normalize=False):
        ot_ps = [ps_o_pool.tile([DP, 512], F32, tag="ot_ps", name=f"ot_ps_{i}") for i in range(2)]
        nkt = 4 * (qb + 1)
        # diagonal k-tiles first: the DVE mask-multiply latency lands in the
        # pipeline fill instead of the steady state
        kt_order = list(range(4 * qb, nkt)) + list(range(0, 4 * qb))
        pts = {}

        def emit_scores_exp(kt):
            tb = kt // 4
            qs = (kt - 4 * qb) * P if kt >= 4 * qb else 0
            nq = 512 - qs
            s_ps = ps_s_pool.tile([P, 2, 512], F32, tag="s_ps", name="s_ps")
            for h2 in range(2):
                # S_T[k, q] for head h = 2*hp + h2 (row-packed pair)
                nc.tensor.matmul(
                    s_ps[:, h2, qs:],
                    lhsT=KT[hp][tb][
                        h2 * D : (h2 + 1) * D,
                        (kt % 4) * P : (kt % 4 + 1) * P,
                    ],
                    rhs=QT[hp][qb][h2 * D : (h2 + 1) * D, qs:],
                    start=True,
                    stop=True,
                )
            p_t = pt_pool.tile([P, 2, 512], F16, tag="pt", name="p_t")
            nc.scalar.activation(
                p_t[:, :, qs:],
                s_ps[:, :, qs:],
                mybir.ActivationFunctionType.Exp,
                scale=0.125,
            )
            if kt >= 4 * qb:  # diagonal: zero q < k entries
                j = kt - 4 * qb
                mb = masks[:, j : j + 1, qs:].to_broadcast([P, 2, nq])
                nc.vector.tensor_mul(p_t[:, :, qs:], p_t[:, :, qs:], mb)
            pts[kt] = (p_t, qs)

        def emit_av(kt, first, last):
            tb = kt // 4
            p_t, qs = pts.pop(kt)
            for h2 in range(2):
                h = 2 * hp + h2
                nc.tensor.matmul(
                    ot_ps[h2][:, qs:],
                    lhsT=V[tb][:, kt % 4, h * DP : (h + 1) * DP],
                    rhs=p_t[:, h2, qs:],
                    start=first,
                    stop=last,
                )

        # software pipeline: S(i+1) before AV(i), one gemm filler per k-tile
        emit_scores_exp(kt_order[0])
        for i in range(1, nkt):
            emit_scores_exp(kt_order[i])
            emit_av(kt_order[i - 1], first=(i == 1), last=False)
            take_filler(1)
        emit_av(kt_order[-1], first=(nkt == 1), last=True)

        # release the AV psum banks immediately; normalize off the critical
        # path from the sbuf copy
        osb = [osb_pool.tile([DP, 512], F32, tag="osb", name="osb") for _ in range(2)]
        for h2 in range(2):
            nc.vector.tensor_copy(osb[h2][:], ot_ps[h2][:])

        def norm_unit(h2):
            def emit():
                recip = recip_pool.tile([1, 512], F32, tag="recip", name="recip")
                nc.vector.tensor_copy(recip[:], osb[h2][D : D + 1, :])
                nc.vector.reciprocal_approx_fast(recip[:], recip[:])
                bc = bc_pool.tile([D, 512], F32, tag="bc", name="bc")
                nc.gpsimd.partition_broadcast(bc[:], recip[:])
                nc.vector.tensor_mul(
                    OT[hp][qb][h2 * D : (h2 + 1) * D, :],
                    osb[h2][:D, :],
                    bc[:],
                )

            return emit

        # normalize is deferred through the filler queue so its DVE/gpsimd
        # chain doesn't sit in front of the next pair's mask-multiplies;
        # the final pairs run it eagerly (the output-proj tail gates on OT)
        if eager_normalize:
            for h2 in range(2):
                norm_unit(h2)()
        else:
            for h2 in range(2):
                fillers.append((False, norm_unit(h2)))

    # ---- schedule ----
    # prologue: qkv(0) straight (nothing to overlap with yet)
    push_qkv(0)
    flush_qkv()
    for qb in range(QB):
        if qb < 3:
            push_qkv(qb + 1)
        if qb >= 1:
            push_proj(qb - 1)
        for hp in range(4):
            attention_pair(qb, hp, eager_normalize=(qb == 3 and hp >= 2))
            take_filler(2)
        flush_qkv()
    flush_all()
    # tail: last 4 proj tiles with 4 psum tiles in flight so only the ct=3
    # matmuls wait on the last pair's normalize
    tail_units = [proj_tile_units(tt, pair_pool=(tt < 14)) for tt in range(12, 16)]
    for tt_i in range(3):  # ct 0..2 of both halves for tiles 12,13,14
        for u in (0, 1, 2, 4, 5, 6):
            tail_units[tt_i][u]()
    for tt_i in range(3):  # ct=3 closes + copy + dma
        tail_units[tt_i][3]()
        tail_units[tt_i][7]()
        tail_units[tt_i][8]()
    for u in tail_units[3]:
        u()

    return nc


_CACHED_NC = None


def get_nc():
    global _CACHED_NC
    if _CACHED_NC is None:
        nc = bacc.Bacc()
        with tile.TileContext(nc) as tc:
            build_attention_kernel(tc)
        nc.compile()
        _CACHED_NC = nc
    return _CACHED_NC


def make_in_maps(x, W_att, W_proj):
    x = np.asarray(x, dtype=np.float32)
    W_att = np.asarray(W_att, dtype=np.float32)
    in_maps = []
    for c in range(NC_CORES):
        b, hg = c // 2, c % 2
        s = hg * 512
        in_maps.append(
            {
                "xT": np.ascontiguousarray(x[b].T).astype(F16_NP),
                "wk": np.ascontiguousarray(
                    W_att[:, 0 * C + s : 0 * C + s + 512]
                ).astype(F16_NP),
                "wq": np.ascontiguousarray(
                    W_att[:, 1 * C + s : 1 * C + s + 512]
                ).astype(F16_NP),
                "wv": np.ascontiguousarray(
                    W_att[:, 2 * C + s : 2 * C + s + 512]
                ).astype(F16_NP),
                "wp": np.ascontiguousarray(
                    np.asarray(W_proj, np.float32)[s : s + 512]
                ).astype(F16_NP),
            }
        )
    return in_maps


def combine_outputs(results, b_proj):
    B = NC_CORES // 2
    out = np.empty((B, T, C), dtype=np.float32)
    bias = np.asarray(b_proj, dtype=np.float32)
    for b in range(B):
        out[b] = (
            results[2 * b]["y"].astype(np.float32)
            + results[2 * b + 1]["y"].astype(np.float32)
            + bias
        )
    return out


def kernel(x, W_att, W_proj, b_proj):
    from concourse.bass_utils import run_bass_kernel_spmd

    nc = get_nc()
    in_maps = make_in_maps(x, W_att, W_proj)
    res = run_bass_kernel_spmd(nc, in_maps, list(range(NC_CORES)))
    return combine_outputs(res.results, b_proj)
